# revision 12
# baseline (speedup 1.0000x reference)
"""Trainium2 Bass kernel for nn_Cabasc (aspect-based sentiment model).

Feature-major GRU design:
  - hidden state kept transposed: h_fm [128, 3chunk x 2side x 16seq]; the
    recurrent matmuls use Whh chunks as the stationary operand (lhsT) and h_fm
    as the 16-col moving operand -> no per-step transposes, tiny PE streams.
  - gi (input gates) accumulated by the production pipeline directly into the
    per-step PSUM gate slots (start of the accumulation group); the per-step
    Whh matmuls continue the same group (start=False).
  - validity masking folded into the embedding table: token 0 (= padding)
    carries +64 on the z-gate preactivation -> z~=1 -> exact state carry.
  - attention dot h.wl via n=1 matmuls into a PSUM strip, flushed every 64
    steps.
"""
import os
os.environ.setdefault("TILE_EXHAUSTIVE_MEMORY_SHARE_CHECK", "1")

import numpy as np
import ml_dtypes

B, L, LA, V, D, H, NP = 128, 512, 5, 32000, 300, 300, 3
NCORES = 8
SPB = 16          # sequences per core
DP = 384          # padded embedding dim (col 300 = 1.0 bias column, col 301 = pad flag)
GP = 1152         # padded gate width: 3 gates x 384

_CACHE = {}


def _build(T, debug=False):
    import contextlib
    import concourse.bass as bass
    import concourse.bacc as bacc
    import concourse.tile as tile
    from concourse import mybir

    bf16 = mybir.dt.bfloat16
    f32 = mybir.dt.float32
    i32 = mybir.dt.int32
    AF = mybir.ActivationFunctionType
    OP = mybir.AluOpType
    AX = mybir.AxisListType

    NG4 = (T + 3) // 4          # one gather / gi production covers 4 steps

    nc = bacc.Bacc("TRN2", target_bir_lowering=False, debug=False)

    # ---------------- DRAM inputs ----------------
    embp = nc.dram_tensor("embp", [V, DP], bf16, kind="ExternalInput")
    wih_l = nc.dram_tensor("wih_l", [DP, GP], bf16, kind="ExternalInput")
    wih_r = nc.dram_tensor("wih_r", [DP, GP], bf16, kind="ExternalInput")
    whh_l = nc.dram_tensor("whh_l", [DP, GP], bf16, kind="ExternalInput")
    whh_r = nc.dram_tensor("whh_r", [DP, GP], bf16, kind="ExternalInput")
    bhhnD = nc.dram_tensor("bhhnD", [128, 96], bf16, kind="ExternalInput")
    wlrD = nc.dram_tensor("wlrD", [128, 6], bf16, kind="ExternalInput")
    wpkT = nc.dram_tensor("wpkT", [DP, DP], bf16, kind="ExternalInput")
    wmT = nc.dram_tensor("wmT", [DP, DP], bf16, kind="ExternalInput")
    wdT = nc.dram_tensor("wdT", [DP, 4], bf16, kind="ExternalInput")
    hv2 = nc.dram_tensor("hv2", [DP, 1], bf16, kind="ExternalInput")
    hv = nc.dram_tensor("hv", [DP, 1], bf16, kind="ExternalInput")
    identD = nc.dram_tensor("identD", [128, 128], bf16, kind="ExternalInput")
    bvecsD = nc.dram_tensor("bvecsD", [DP, 3], f32, kind="ExternalInput")
    bdD = nc.dram_tensor("bdD", [4, 1], f32, kind="ExternalInput")
    sigbD = nc.dram_tensor("sigbD", [SPB, 2], f32, kind="ExternalInput")
    rmlD = nc.dram_tensor("rmlD", [SPB, 1], f32, kind="ExternalInput")
    ralD = nc.dram_tensor("ralD", [1, SPB], f32, kind="ExternalInput")
    qacD = nc.dram_tensor("qacD", [SPB, 1], f32, kind="ExternalInput")
    masksD = nc.dram_tensor("masksD", [SPB, 3, L], bf16, kind="ExternalInput")
    gidxD = nc.dram_tensor("gidxD", [128, NG4], i32, kind="ExternalInput")
    midxD = nc.dram_tensor("midxD", [128, 64], i32, kind="ExternalInput")
    aidxD = nc.dram_tensor("aidxD", [128, 1], i32, kind="ExternalInput")
    smatD = nc.dram_tensor("smatD", [SPB, 128, 4, L], bf16, kind="ExternalInput")

    out_probs = nc.dram_tensor("out_probs", [SPB, 3], f32, kind="ExternalOutput")
    if debug:
        dbg_h = nc.dram_tensor("dbg_h", [128, 96], f32, kind="ExternalOutput")
        dbg_attn = nc.dram_tensor("dbg_attn", [SPB, 2, T], f32, kind="ExternalOutput")
        dbg_w = nc.dram_tensor("dbg_w", [SPB, L], f32, kind="ExternalOutput")

    with tile.TileContext(nc) as tc:
        outer = contextlib.ExitStack()
        with outer:
            con = outer.enter_context(tc.tile_pool(name="con", bufs=1))
            sb = outer.enter_context(tc.tile_pool(name="sb", bufs=1))

            # ---------------- constants ----------------
            ident = con.tile([128, 128], bf16)
            nc.sync.dma_start(ident[:], identD[:])
            wih_t = [[con.tile([128, GP], bf16, name=f"wih{s}{k}") for k in range(3)]
                     for s in range(2)]
            whh_t = [[con.tile([128, GP], bf16, name=f"whh{s}{k}") for k in range(3)]
                     for s in range(2)]
            for s, src_ih, src_hh in ((0, wih_l, whh_l), (1, wih_r, whh_r)):
                for k in range(3):
                    nc.sync.dma_start(wih_t[s][k][:], src_ih[128 * k:128 * (k + 1), :])
                    nc.sync.dma_start(whh_t[s][k][:], src_hh[128 * k:128 * (k + 1), :])
            bhhn_t = con.tile([128, 96], bf16)
            nc.sync.dma_start(bhhn_t[:], bhhnD[:])
            wlr_t = con.tile([128, 6], bf16)
            nc.sync.dma_start(wlr_t[:], wlrD[:])
            sigb_t = con.tile([SPB, 2], f32)
            nc.sync.dma_start(sigb_t[:], sigbD[:])
            gidx_t = con.tile([128, NG4], i32)
            nc.sync.dma_start(gidx_t[:], gidxD[:])
            midx_t = con.tile([128, 64], i32)
            nc.sync.dma_start(midx_t[:], midxD[:])
            aidx_t = con.tile([128, 1], i32)
            nc.sync.dma_start(aidx_t[:], aidxD[:])
            masks_t = con.tile([SPB, 3, L], bf16)
            nc.sync.dma_start(masks_t[:], masksD[:])
            wpkT_t = [con.tile([128, DP], bf16, name=f"wpkT{k}") for k in range(3)]
            wmT_t = [con.tile([128, DP], bf16, name=f"wmT{k}") for k in range(3)]
            wdT_t = [con.tile([128, 4], bf16, name=f"wdT{k}") for k in range(3)]
            hv2_t = [con.tile([128, 1], bf16, name=f"hv2{k}") for k in range(3)]
            hv_t = [con.tile([128, 1], bf16, name=f"hv{k}") for k in range(3)]
            bvec_t = [con.tile([128, 3], f32, name=f"bvec{k}") for k in range(3)]
            for k in range(3):
                o = 128 * k
                nc.sync.dma_start(wpkT_t[k][:], wpkT[o:o + 128, :])
                nc.sync.dma_start(wmT_t[k][:], wmT[o:o + 128, :])
                nc.sync.dma_start(wdT_t[k][:], wdT[o:o + 128, :])
                nc.sync.dma_start(hv2_t[k][:], hv2[o:o + 128, :])
                nc.sync.dma_start(hv_t[k][:], hv[o:o + 128, :])
                nc.sync.dma_start(bvec_t[k][:], bvecsD[o:o + 128, :])
            bd_t = con.tile([4, 1], f32)
            nc.sync.dma_start(bd_t[:], bdD[:])
            rml_t = con.tile([SPB, 1], f32)
            nc.sync.dma_start(rml_t[:], rmlD[:])
            ral_t = con.tile([1, SPB], f32)
            nc.sync.dma_start(ral_t[:], ralD[:])
            qac_t = con.tile([SPB, 1], f32)
            nc.sync.dma_start(qac_t[:], qacD[:])

            # ---------------- persistent state ----------------
            # h_fm[p, 32c+16s+b] = h_{side s, seq b}[128c+p]  (c2 rows 44.. = 0)
            h_fm = sb.tile([128, 96], bf16)
            nc.vector.memset(h_fm[:], 0.0)
            attn_sb = sb.tile([SPB, 2, T], f32)      # h.wl / h.wr dots per step
            ex_sb = [sb.tile([128, 3, 128], bf16, name=f"ex{i}") for i in range(2)]
            stg_n = [sb.tile([128, 4, 96], bf16, name=f"sgn{i}") for i in range(2)]

            # =============== GRU phase ===============
            gru = contextlib.ExitStack()
            with gru:
                work = gru.enter_context(tc.tile_pool(name="work", bufs=2))
                ps_g = gru.enter_context(tc.tile_pool(name="ps_g", bufs=2, space="PSUM"))
                ps_n = gru.enter_context(tc.tile_pool(name="ps_n", bufs=1, space="PSUM"))
                ps_e = gru.enter_context(tc.tile_pool(name="ps_e", bufs=1, space="PSUM"))
                ps_a = gru.enter_context(tc.tile_pool(name="ps_a", bufs=1, space="PSUM"))
                ps_gn = gru.enter_context(tc.tile_pool(name="ps_gn", bufs=1, space="PSUM"))

                # pg_rz[p, s4, 96g+32mc+16side+b]: r/z gate preacts, 4 steps
                pg_rz = [ps_g.tile([128, 4, 192], f32, name=f"pgrz{i}", tag="pgrz")
                         for i in range(2)]
                pg_n = ps_n.tile([128, 4, 96], f32, name="pgn", tag="pgn")
                ps_ex = ps_e.tile([128, 3, 128], bf16, name="psex", tag="psex")
                attn_ps = ps_a.tile([SPB, 2, 64], f32, name="attnps", tag="attnps")
                gates_n = ps_gn.tile([128, 96], f32, name="gatesn", tag="gatesn")

                def prod_gather(g):
                    rows = work.tile([128, DP], bf16, name="gthr", tag="gthr")
                    nc.gpsimd.indirect_dma_start(
                        out=rows[:], out_offset=None, in_=embp[:],
                        in_offset=bass.IndirectOffsetOnAxis(ap=gidx_t[:, g:g + 1], axis=0))
                    return rows

                def prod_transpose(g, rows):
                    for c in range(3):
                        nc.tensor.transpose(ps_ex[:, c, :], rows[:, 128 * c:128 * (c + 1)],
                                            ident[:])
                    nc.scalar.activation(ex_sb[g % 2][:], ps_ex[:], AF.Copy)

                def prod_mm(g, part):
                    # gi matmuls: out[128, (4 steps, 16 b)] over 64 tokens of one side
                    ex = ex_sb[g % 2]
                    prz = pg_rz[g % 2]
                    tasks = [(gg, mc, s) for gg in range(3) for mc in range(3)
                             for s in range(2)]
                    lo, hi = (0, 9) if part == 0 else (9, 18)
                    for (gg, mc, s) in tasks[lo:hi]:
                        for kc in range(3):
                            if gg < 2:
                                out = prz[:, :, 96 * gg + 32 * mc + 16 * s:
                                          96 * gg + 32 * mc + 16 * s + 16]
                            else:
                                out = pg_n[:, :, 32 * mc + 16 * s:32 * mc + 16 * s + 16]
                            nc.tensor.matmul(
                                out,
                                lhsT=wih_t[s][kc][:, 384 * gg + 128 * mc:
                                                  384 * gg + 128 * (mc + 1)],
                                rhs=ex[:, kc, 64 * s:64 * s + 64],
                                start=(kc == 0), stop=False,
                                skip_group_check=True)

                def prod_ncopy(g):
                    nc.scalar.activation(stg_n[g % 2][:], pg_n[:], AF.Copy)

                # preamble: produce group 0 fully
                rows0 = prod_gather(0)
                prod_transpose(0, rows0)
                prod_mm(0, 0)
                prod_mm(0, 1)
                prod_ncopy(0)

                for t in range(T):
                    s4 = t % 4
                    gg = t // 4
                    gp = gg + 1     # group being produced during this window
                    prz = pg_rz[gg % 2]

                    # ---- recurrent matmuls (continue the gi groups) ----
                    for gate in range(2):           # r then z
                        for mc in range(3):
                            for side in range(2):
                                col = 96 * gate + 32 * mc + 16 * side
                                for kc in range(3):
                                    nc.tensor.matmul(
                                        prz[:, s4, col:col + 16],
                                        lhsT=whh_t[side][kc][:, 384 * gate + 128 * mc:
                                                             384 * gate + 128 * (mc + 1)],
                                        rhs=h_fm[:, 32 * kc + 16 * side:
                                                 32 * kc + 16 * side + 16],
                                        start=False, stop=(kc == 2),
                                        skip_group_check=True)
                    # n-gate: bhh_n init + Whh_n accumulation (no gi here)
                    for mc in range(3):
                        for side in range(2):
                            col = 32 * mc + 16 * side
                            nc.tensor.matmul(
                                gates_n[0:128, col:col + 16],
                                lhsT=ident[:, 0:128],
                                rhs=bhhn_t[:, col:col + 16],
                                start=True, stop=False, skip_group_check=True)
                            for kc in range(3):
                                nc.tensor.matmul(
                                    gates_n[0:128, col:col + 16],
                                    lhsT=whh_t[side][kc][:, 768 + 128 * mc:
                                                         768 + 128 * (mc + 1)],
                                    rhs=h_fm[:, 32 * kc + 16 * side:
                                             32 * kc + 16 * side + 16],
                                    start=False, stop=(kc == 2),
                                    skip_group_check=True)

                    # ---- gate nonlinearities / state update ----
                    r_sb = work.tile([128, 96], bf16, name="r_sb", tag="r_sb")
                    nc.scalar.activation(r_sb[:], prz[:, s4, 0:96], AF.Sigmoid)
                    zc_sb = work.tile([128, 96], bf16, name="zc_sb", tag="zc_sb")
                    nc.scalar.activation(zc_sb[:], prz[:, s4, 96:192], AF.Sigmoid,
                                         scale=-1.0)
                    t1 = work.tile([128, 96], bf16, name="t1", tag="t1")
                    nc.vector.tensor_tensor(out=t1[:], in0=r_sb[:], in1=gates_n[:],
                                            op=OP.mult)
                    t2 = work.tile([128, 96], bf16, name="t2", tag="t2")
                    nc.vector.tensor_tensor(out=t2[:], in0=t1[:],
                                            in1=stg_n[gg % 2][:, s4, :], op=OP.add)
                    n_sb = work.tile([128, 96], bf16, name="n_sb", tag="n_sb")
                    nc.scalar.activation(n_sb[:], t2[:], AF.Tanh)
                    ta = work.tile([128, 96], bf16, name="ta", tag="ta")
                    nc.vector.tensor_tensor(out=ta[:], in0=zc_sb[:], in1=h_fm[:],
                                            op=OP.mult)
                    tb = work.tile([128, 96], bf16, name="tb", tag="tb")
                    nc.vector.tensor_tensor(out=tb[:], in0=h_fm[:], in1=ta[:],
                                            op=OP.subtract)
                    tcm = work.tile([128, 96], bf16, name="tcm", tag="tcm")
                    nc.vector.tensor_tensor(out=tcm[:], in0=zc_sb[:], in1=n_sb[:],
                                            op=OP.mult)
                    nc.vector.tensor_tensor(out=h_fm[:], in0=tb[:], in1=tcm[:],
                                            op=OP.add)

                    # ---- attention dot: attn[b, side, t] = h_side_b . w_side ----
                    w64 = t % 64
                    for side in range(2):
                        for c in range(3):
                            nc.tensor.matmul(
                                attn_ps[:, side, w64:w64 + 1],
                                lhsT=h_fm[:, 32 * c + 16 * side:32 * c + 16 * side + 16],
                                rhs=wlr_t[:, 2 * c + side:2 * c + side + 1],
                                start=(c == 0), stop=(c == 2),
                                skip_group_check=True)
                    if w64 == 63 or t == T - 1:
                        t0c = t - w64
                        nc.scalar.activation(attn_sb[:, :, t0c:t + 1],
                                             attn_ps[:, :, 0:w64 + 1], AF.Copy)

                    # ---- spread production of group gp ----
                    if gp < NG4:
                        if s4 == 0:
                            work._prod_rows = prod_gather(gp)
                        elif s4 == 1:
                            prod_transpose(gp, work._prod_rows)
                        elif s4 == 2:
                            prod_mm(gp, 0)
                        else:
                            prod_mm(gp, 1)
                            prod_ncopy(gp)

            if debug:
                dbgh = sb.tile([128, 96], f32)
                nc.vector.tensor_copy(dbgh[:], h_fm[:])
                nc.sync.dma_start(dbg_h[:], dbgh[:])
                nc.sync.dma_start(dbg_attn[:], attn_sb[:])

            # =============== post phase ===============
            post = contextlib.ExitStack()
            with post:
                pp = post.enter_context(tc.tile_pool(name="pp", bufs=1))
                pw = post.enter_context(tc.tile_pool(name="pw", bufs=2))
                ps_pa = post.enter_context(tc.tile_pool(name="ps_pa", bufs=2, space="PSUM"))
                ps_pb = post.enter_context(tc.tile_pool(name="ps_pb", bufs=1, space="PSUM"))
                ps_pc = post.enter_context(tc.tile_pool(name="ps_pc", bufs=2, space="PSUM"))

                # ---- attn sigmoid + 0.5 ----
                af = pp.tile([SPB, 2, L], bf16)
                nc.vector.memset(af[:], 0.0)
                af_f = pp.tile([SPB, 2, T], f32)
                for side in range(2):
                    nc.scalar.activation(af_f[:, side, :], attn_sb[:, side, :],
                                         AF.Sigmoid, bias=sigb_t[:, side:side + 1])
                nc.vector.tensor_scalar_add(af[:, :, 0:T], af_f[:], 0.5)

                # ---- shifted attn_r via per-seq shifted-identity matmul ----
                # arT[j, q, b] = af[b, 1, 128q+j]
                arT = pp.tile([128, 4, SPB], bf16)
                for q in range(4):
                    pta = ps_pa.tile([128, 128], bf16, name="pta", tag="pta")
                    nc.tensor.transpose(pta[0:128, 0:16], af[:, 1, 128 * q:128 * (q + 1)],
                                        ident[0:16, 0:16])
                    nc.scalar.activation(arT[:, q, :], pta[0:128, 0:16], AF.Copy)
                wsh = pp.tile([SPB, L], bf16)
                for b in range(SPB):
                    smat_t = pw.tile([128, 4, L], bf16, name="smat_t", tag="smat")
                    nc.sync.dma_start(smat_t[:], smatD[b, :, :, :])
                    psh = ps_pc.tile([1, L], f32, name="psh", tag="ps0")
                    for q in range(4):
                        nc.tensor.matmul(psh[:], lhsT=arT[:, q:q + 1, b:b + 1],
                                         rhs=smat_t[:, q, :],
                                         start=(q == 0), stop=(q == 3))
                    shrow = pw.tile([1, L], bf16, name="shrow", tag="shrow")
                    nc.scalar.activation(shrow[:], psh[:], AF.Copy)
                    nc.sync.dma_start(wsh[b:b + 1, :], shrow[:])

                # ---- w combine ----
                w1 = pp.tile([SPB, L], bf16)
                nc.vector.tensor_tensor(out=w1[:], in0=af[:, 0, :], in1=masks_t[:, 0, :],
                                        op=OP.mult)
                w2 = pp.tile([SPB, L], bf16)
                nc.vector.tensor_tensor(out=w2[:], in0=wsh[:], in1=masks_t[:, 1, :],
                                        op=OP.mult)
                wt_ = pp.tile([SPB, L], bf16)
                nc.vector.tensor_tensor(out=wt_[:], in0=w1[:], in1=w2[:], op=OP.add)
                nc.vector.tensor_tensor(out=wt_[:], in0=wt_[:], in1=masks_t[:, 2, :],
                                        op=OP.add)
                w_n = pp.tile([SPB, L], bf16)
                nc.vector.tensor_scalar_mul(w_n[:], wt_[:], rml_t[:, 0:1])
                if debug:
                    dbgw = pp.tile([SPB, L], f32)
                    nc.vector.tensor_copy(dbgw[:], wt_[:])
                    nc.sync.dma_start(dbg_w[:], dbgw[:])

                # ---- aspect -> score bias (qa + bk.wa1) ----
                arows = pp.tile([128, DP], bf16)
                nc.gpsimd.indirect_dma_start(
                    out=arows[:], out_offset=None, in_=embp[:],
                    in_offset=bass.IndirectOffsetOnAxis(ap=aidx_t[:, 0:1], axis=0))
                aspsum = pp.tile([128, 3 * SPB], f32)
                for c in range(3):
                    pta = ps_pa.tile([128, 128], bf16, name="pta2", tag="pta")
                    nc.tensor.transpose(pta[:], arows[:, 128 * c:128 * (c + 1)], ident[:])
                    aT = pw.tile([128, 128], f32, name="aT", tag="aT")
                    nc.vector.tensor_copy(aT[:], pta[:])
                    for b in range(SPB):
                        nc.vector.tensor_reduce(
                            aspsum[:, c * SPB + b:c * SPB + b + 1],
                            aT[:, 8 * b:8 * b + 5], axis=AX.X, op=OP.add)
                aspb = pp.tile([128, 3 * SPB], bf16)
                nc.vector.tensor_copy(aspb[:], aspsum[:])
                pqa = ps_pc.tile([1, SPB], f32, name="pqa", tag="small")
                for c in range(3):
                    nc.tensor.matmul(pqa[:], lhsT=hv_t[c][:, 0:1],
                                     rhs=aspb[:, c * SPB:(c + 1) * SPB],
                                     start=(c == 0), stop=(c == 2))
                sc_bias = pp.tile([1, SPB], f32)
                nc.vector.tensor_tensor(out=sc_bias[:], in0=pqa[:], in1=ral_t[:],
                                        op=OP.mult)
                nc.vector.tensor_scalar_add(sc_bias[:], sc_bias[:], qac_t[0:1, 0:1])

                # ---- wn column transposes (wnT[p, q, b] = w_n[b, 128q+p]) ----
                wnT = pp.tile([128, 4, SPB], bf16)
                for q in range(4):
                    ptw = ps_pa.tile([128, 128], bf16, name="ptw", tag="pta")
                    nc.tensor.transpose(ptw[0:128, 0:16], w_n[:, 128 * q:128 * (q + 1)],
                                        ident[0:16, 0:16])
                    nc.scalar.activation(wnT[:, q, :], ptw[0:128, 0:16], AF.Copy)

                # ---- per-seq memory pipeline ----
                # mw = sum_l cv[l]*raw[l,:], vs = sum_l wn[l]*raw[l,:] via n=1 matmuls
                uts_ps = ps_pb.tile([128, 3 * SPB], f32, name="utsps", tag="utsps")
                vs_ps = ps_pb.tile([128, 3 * SPB], f32, name="vsps", tag="vsps")
                for b in range(SPB):
                    mrows = [pw.tile([128, DP], bf16, name=f"mr{q}", tag=f"mr{q}")
                             for q in range(4)]
                    for q in range(4):
                        nc.gpsimd.indirect_dma_start(
                            out=mrows[q][:], out_offset=None, in_=embp[:],
                            in_offset=bass.IndirectOffsetOnAxis(
                                ap=midx_t[:, 4 * b + q:4 * b + q + 1], axis=0))
                    emT = pw.tile([128, 3, L], bf16, name="emT", tag="emT")
                    for q in range(4):
                        for c in range(3):
                            ptm = ps_pa.tile([128, 128], bf16, name="ptm", tag="pta")
                            nc.tensor.transpose(ptm[:], mrows[q][:, 128 * c:128 * (c + 1)],
                                                ident[:])
                            nc.scalar.activation(emT[:, c, 128 * q:128 * (q + 1)],
                                                 ptm[:], AF.Copy)
                    # s0[l] = raw[l,:].hv2  (hv2 = Wk^T w_att[:D]; bk.wa1 in sc_bias)
                    ps0 = ps_pc.tile([1, L], f32, name="ps0", tag="ps0")
                    for c in range(3):
                        nc.tensor.matmul(ps0[:], lhsT=hv2_t[c][:, 0:1], rhs=emT[:, c, :],
                                         start=(c == 0), stop=(c == 2))
                    s0row = pw.tile([1, L], f32, name="s0row", tag="s0row")
                    nc.scalar.activation(s0row[:], ps0[:], AF.Copy)
                    # score & softmax for seq b, all on partition 0
                    wrow = pw.tile([1, L], bf16, name="wrow", tag="wrow")
                    nc.sync.dma_start(wrow[:], wt_[b:b + 1, :])
                    spre = pw.tile([1, L], f32, name="spre", tag="spre")
                    nc.vector.tensor_tensor(out=spre[:], in0=wrow[:], in1=s0row[:],
                                            op=OP.mult)
                    score = pw.tile([1, L], f32, name="score", tag="score")
                    nc.scalar.activation(score[:], spre[:], AF.Tanh,
                                         bias=sc_bias[0:1, b:b + 1])
                    mneg = pw.tile([1, 1], f32, name="mneg", tag="mneg")
                    nc.vector.tensor_reduce(mneg[:], score[:], axis=AX.X, op=OP.max,
                                            negate=True)
                    ex_t = pw.tile([1, L], f32, name="ex_t", tag="ex_t")
                    zsum = pw.tile([1, 1], f32, name="zsum", tag="zsum")
                    nc.scalar.activation(ex_t[:], score[:], AF.Exp, bias=mneg[0:1, 0:1],
                                         accum_out=zsum[0:1, 0:1])
                    zrec = pw.tile([1, 1], f32, name="zrec", tag="zrec")
                    nc.vector.reciprocal(zrec[:], zsum[:])
                    prob = pw.tile([1, L], f32, name="prob", tag="prob")
                    nc.vector.tensor_scalar_mul(prob[:], ex_t[:], zrec[0:1, 0:1])
                    cvrow = pw.tile([1, L], bf16, name="cvrow", tag="cvrow")
                    nc.vector.tensor_tensor(out=cvrow[:], in0=prob[:], in1=wrow[:],
                                            op=OP.mult)
                    # cv column form via 4 tiny transposes
                    ptc = ps_pa.tile([128, 64, 2], bf16, name="ptc", tag="pta")
                    for q in range(4):
                        nc.tensor.transpose(ptc[0:128, q, 0:1],
                                            cvrow[0:1, 128 * q:128 * (q + 1)],
                                            ident[0:1, 0:1])
                    cvT = pw.tile([128, 4], bf16, name="cvT", tag="cvT")
                    nc.scalar.activation(cvT[:], ptc[0:128, 0:4, 0], AF.Copy)
                    for c in range(3):
                        for q in range(4):
                            nc.tensor.matmul(
                                uts_ps[:, c * SPB + b:c * SPB + b + 1],
                                lhsT=mrows[q][:, 128 * c:128 * (c + 1)],
                                rhs=cvT[:, q:q + 1],
                                start=(q == 0), stop=(q == 3), skip_group_check=True)
                            nc.tensor.matmul(
                                vs_ps[:, c * SPB + b:c * SPB + b + 1],
                                lhsT=mrows[q][:, 128 * c:128 * (c + 1)],
                                rhs=wnT[:, q, b:b + 1],
                                start=(q == 0), stop=(q == 3), skip_group_check=True)

                # ---- Wpk.mw + bpk + v_s -> Wm/tanh -> Wd -> softmax ----
                utsb = pp.tile([128, 3 * SPB], bf16)
                nc.vector.tensor_copy(utsb[:], uts_ps[:])
                vsT = pp.tile([128, 3 * SPB], f32)
                nc.scalar.activation(vsT[:], vs_ps[:], AF.Copy)
                vns = pp.tile([128, 3 * SPB], bf16)
                for oc in range(3):
                    pv = ps_pc.tile([128, SPB], f32, name="pv", tag="small")
                    for ic in range(3):
                        nc.tensor.matmul(pv[:],
                                         lhsT=wpkT_t[ic][:, 128 * oc:128 * (oc + 1)],
                                         rhs=utsb[:, ic * SPB:(ic + 1) * SPB],
                                         start=(ic == 0), stop=(ic == 2))
                    nc.vector.scalar_tensor_tensor(
                        out=vns[:, oc * SPB:(oc + 1) * SPB], in0=pv[:],
                        scalar=bvec_t[oc][:, 1:2], in1=vsT[:, oc * SPB:(oc + 1) * SPB],
                        op0=OP.add, op1=OP.add)
                vms = pp.tile([128, 3 * SPB], bf16)
                for oc in range(3):
                    pv2 = ps_pc.tile([128, SPB], f32, name="pv2", tag="small")
                    for ic in range(3):
                        nc.tensor.matmul(pv2[:],
                                         lhsT=wmT_t[ic][:, 128 * oc:128 * (oc + 1)],
                                         rhs=vns[:, ic * SPB:(ic + 1) * SPB],
                                         start=(ic == 0), stop=(ic == 2))
                    nc.scalar.activation(vms[:, oc * SPB:(oc + 1) * SPB], pv2[:],
                                         AF.Tanh, bias=bvec_t[oc][:, 2:3])
                plg = ps_pc.tile([4, SPB], f32, name="plg", tag="small")
                for ic in range(3):
                    nc.tensor.matmul(plg[:], lhsT=wdT_t[ic][:, 0:4],
                                     rhs=vms[:, ic * SPB:(ic + 1) * SPB],
                                     start=(ic == 0), stop=(ic == 2))
                lgb = pp.tile([4, SPB], bf16)
                nc.vector.tensor_scalar_add(lgb[:], plg[:], bd_t[0:4, 0:1])
                plt = ps_pc.tile([SPB, 4], bf16, name="plt", tag="small")
                nc.tensor.matmul(plt[:], lhsT=lgb[:], rhs=ident[0:4, 0:4],
                                 start=True, stop=True, is_transpose=True)
                mneg2 = pp.tile([SPB, 1], f32)
                nc.vector.tensor_reduce(mneg2[:], plt[:, 0:3], axis=AX.X, op=OP.max,
                                        negate=True)
                ex2 = pp.tile([SPB, 3], f32)
                z2 = pp.tile([SPB, 1], f32)
                nc.scalar.activation(ex2[:], plt[:, 0:3], AF.Exp, bias=mneg2[:, 0:1],
                                     accum_out=z2[:, 0:1])
                z2r = pp.tile([SPB, 1], f32)
                nc.vector.reciprocal(z2r[:], z2[:])
                res = pp.tile([SPB, 3], f32)
                nc.vector.tensor_scalar_mul(res[:], ex2[:], z2r[:, 0:1])
                nc.sync.dma_start(out_probs[:], res[:])

    nc.compile()
    return nc


def _host_prep(inputs, T_override=None):
    bf = ml_dtypes.bfloat16
    emb = np.asarray(inputs['embedding'], np.float32)
    ti = np.asarray(inputs['text_raw_indices'])
    ai = np.asarray(inputs['aspect_indices'])
    xl = np.asarray(inputs['x_l'])
    xr = np.asarray(inputs['x_r'])
    mem_len = (ti != 0).sum(-1).astype(np.int64)
    asp_len = (ai != 0).sum(-1).astype(np.int64)
    left_len = (xl != 0).sum(-1).astype(np.int64)
    right_len = (xr != 0).sum(-1).astype(np.int64)
    T = int(max(left_len.max(), right_len.max()))
    if T_override is not None:
        T = T_override

    embp = np.zeros((V, DP), np.float32)
    embp[:, :D] = emb
    embp[:, D] = 1.0
    embp[0, D + 1] = 1.0          # pad-token flag -> z-gate kill
    embp = embp.astype(bf)

    LARGE = 64.0

    def aug_ih(Wih, bih, bhh):
        # [DP, GP]: row i<300 = Wih^T; row 300 = bih (+bhh for r/z);
        # row 301 = +LARGE on z block (pad-token carry)
        Wih = np.asarray(Wih, np.float32)
        bih = np.asarray(bih, np.float32)
        bhh = np.asarray(bhh, np.float32)
        a = np.zeros((DP, GP), np.float32)
        for g in range(3):
            a[:D, 384 * g:384 * g + D] = Wih[D * g:D * (g + 1), :].T
            bb = bih[D * g:D * (g + 1)].copy()
            if g < 2:
                bb += bhh[D * g:D * (g + 1)]
            a[D, 384 * g:384 * g + D] = bb
        a[D + 1, 384:384 + D] = LARGE
        return a.astype(bf)

    def aug_hh(Whh):
        Whh = np.asarray(Whh, np.float32)
        a = np.zeros((DP, GP), np.float32)
        for g in range(3):
            a[:D, 384 * g:384 * g + D] = Whh[D * g:D * (g + 1), :].T
        return a.astype(bf)

    def padT(Wsq):
        a = np.zeros((DP, DP), np.float32)
        a[:D, :D] = np.asarray(Wsq, np.float32).T
        return a.astype(bf)

    bhhn = np.zeros((128, 96), np.float32)
    for side, key in ((0, 'bhh_l'), (1, 'bhh_r')):
        bh = np.asarray(inputs[key], np.float32)[2 * D:3 * D]
        for c in range(3):
            nrows = min(128, D - 128 * c)
            for bcol in range(SPB):
                bhhn[:nrows, 32 * c + 16 * side + bcol] = bh[128 * c:128 * c + nrows]
    wlr = np.zeros((128, 6), np.float32)
    for side, key in ((0, 'wl'), (1, 'wr')):
        wv = np.asarray(inputs[key], np.float32)[0]
        for c in range(3):
            nrows = min(128, D - 128 * c)
            wlr[:nrows, 2 * c + side] = wv[128 * c:128 * c + nrows]

    wa = np.asarray(inputs['w_att'], np.float32)
    wdT = np.zeros((DP, 4), np.float32)
    wdT[:D, :3] = np.asarray(inputs['Wd'], np.float32).T
    Wk_f = np.asarray(inputs['Wk'], np.float32)
    Wproj_f = np.asarray(inputs['Wproj'], np.float32)
    bk_f = np.asarray(inputs['bk'], np.float32)
    bvecs = np.zeros((DP, 3), np.float32)
    bvecs[:D, 1] = Wproj_f @ bk_f + np.asarray(inputs['bproj'], np.float32)
    bvecs[:D, 2] = np.asarray(inputs['bm'], np.float32)
    shared = {
        'embp': embp,
        'wih_l': aug_ih(inputs['Wih_l'], inputs['bih_l'], inputs['bhh_l']),
        'wih_r': aug_ih(inputs['Wih_r'], inputs['bih_r'], inputs['bhh_r']),
        'whh_l': aug_hh(inputs['Whh_l']),
        'whh_r': aug_hh(inputs['Whh_r']),
        'bhhnD': bhhn.astype(bf),
        'wlrD': wlr.astype(bf),
        'wpkT': padT(Wproj_f @ Wk_f),
        'wmT': padT(inputs['Wm']),
        'wdT': wdT.astype(bf),
        'hv2': np.concatenate([Wk_f.T @ wa[:D],
                               np.zeros(DP - D, np.float32)])[:, None].astype(bf),
        'hv': np.concatenate([np.asarray(inputs['Wq'], np.float32).T @ wa[D:],
                              np.zeros(DP - D, np.float32)])[:, None].astype(bf),
        'identD': np.eye(128, dtype=np.float32).astype(bf),
        'bvecsD': bvecs,
        'bdD': np.concatenate([np.asarray(inputs['bd'], np.float32),
                               [0.0]])[:, None].astype(np.float32),
    }
    qa_c = float(np.asarray(inputs['bq'], np.float32) @ wa[D:]) + \
        float(np.asarray(inputs['bk'], np.float32) @ wa[:D])

    NG4 = (T + 3) // 4
    per_core = []
    for c in range(NCORES):
        sl = slice(c * SPB, (c + 1) * SPB)
        xlc, xrc = xl[sl], xr[sl]
        mlc, alc = mem_len[sl], asp_len[sl]
        llc = left_len[sl]
        a_start = (llc - alc).astype(np.int64)

        gidx = np.zeros((128, NG4), np.int32)
        for g in range(NG4):
            for s in range(4):
                t = 4 * g + s
                if t >= T:
                    continue
                gidx[16 * s:16 * s + 16, g] = xlc[:, t]
                gidx[64 + 16 * s:64 + 16 * s + 16, g] = xrc[:, t]
        midx = np.zeros((128, 64), np.int32)
        for b in range(SPB):
            for q in range(4):
                midx[:, 4 * b + q] = ti[sl][b, 128 * q:128 * (q + 1)]
        aidx = np.zeros((128, 1), np.int32)
        for b in range(SPB):
            aidx[8 * b:8 * b + 5, 0] = ai[sl][b, :]

        idxL = np.arange(L)[None, :]
        mL = (idxL < llc[:, None]).astype(np.float32)
        mR = ((idxL >= a_start[:, None]) & (idxL < mlc[:, None])).astype(np.float32)
        mP = (idxL >= mlc[:, None]).astype(np.float32)
        masks = np.stack([mL, mR, mP], axis=1).astype(bf)

        smat = np.zeros((SPB, 128, 4, L), np.float32)
        for b in range(SPB):
            s = int(a_start[b])
            jj = np.arange(L - s)
            smat[b, jj % 128, jj // 128, jj + s] = 1.0
        sig_b = np.zeros((SPB, 2), np.float32)
        sig_b[:, 0] = float(np.asarray(inputs['bl'])[0])
        sig_b[:, 1] = float(np.asarray(inputs['br'])[0])

        pc = dict(shared)
        pc.update({
            'gidxD': gidx, 'midxD': midx, 'aidxD': aidx,
            'masksD': masks,
            'sigbD': sig_b,
            'rmlD': (1.0 / mlc.astype(np.float32))[:, None],
            'ralD': (1.0 / alc.astype(np.float32))[None, :],
            'qacD': np.full((SPB, 1), qa_c, np.float32),
            'smatD': smat.astype(bf),
        })
        per_core.append(pc)
    return T, per_core


def kernel(**inputs):
    from concourse.bass_utils import run_bass_kernel_spmd
    T, per_core = _host_prep(inputs)
    key = ("v2", T)
    if key not in _CACHE:
        _CACHE[key] = _build(T, debug=False)
    nc = _CACHE[key]
    res = run_bass_kernel_spmd(nc, per_core, list(range(NCORES)))
    out = np.zeros((B, NP), np.float32)
    for c in range(NCORES):
        out[c * SPB:(c + 1) * SPB, :] = res.results[c]["out_probs"]
    return out


# revision 34
# speedup vs baseline: 1.1167x; 1.1167x over previous
"""Trainium2 Bass kernel for nn_Cabasc (aspect-based sentiment model).

Feature-major GRU design:
  - hidden state kept transposed: h_fm [128, 3chunk x 2side x 16seq]; the
    recurrent matmuls use Whh chunks as the stationary operand (lhsT) and h_fm
    as the 16-col moving operand -> no per-step transposes, tiny PE streams.
  - gi (input gates) accumulated by the production pipeline directly into the
    per-step PSUM gate slots (start of the accumulation group); the per-step
    Whh matmuls continue the same group (start=False).
  - validity masking folded into the embedding table: token 0 (= padding)
    carries +64 on the z-gate preactivation -> z~=1 -> exact state carry.
  - attention dot h.wl via n=1 matmuls into a PSUM strip, flushed every 64
    steps.
"""
import os
os.environ.setdefault("TILE_EXHAUSTIVE_MEMORY_SHARE_CHECK", "1")

import numpy as np
import ml_dtypes

B, L, LA, V, D, H, NP = 128, 512, 5, 32000, 300, 300, 3
NCORES = 8
SPB = 16          # sequences per core
DP = 384          # padded embedding dim (col 300 = 1.0 bias column, col 301 = pad flag)
GP = 1152         # padded gate width: 3 gates x 384

_CACHE = {}


def _build(T, debug=False):
    import contextlib
    import concourse.bass as bass
    import concourse.bacc as bacc
    import concourse.tile as tile
    from concourse import mybir

    bf16 = mybir.dt.bfloat16
    f32 = mybir.dt.float32
    i32 = mybir.dt.int32
    AF = mybir.ActivationFunctionType
    OP = mybir.AluOpType
    AX = mybir.AxisListType

    NG4 = (T + 3) // 4          # one gather / gi production covers 4 steps

    nc = bacc.Bacc("TRN2", target_bir_lowering=False, debug=False)

    # ---------------- DRAM inputs ----------------
    embp = nc.dram_tensor("embp", [V, DP], bf16, kind="ExternalInput")
    wih_l = nc.dram_tensor("wih_l", [DP, GP], bf16, kind="ExternalInput")
    wih_r = nc.dram_tensor("wih_r", [DP, GP], bf16, kind="ExternalInput")
    whh_l = nc.dram_tensor("whh_l", [DP, GP], bf16, kind="ExternalInput")
    whh_r = nc.dram_tensor("whh_r", [DP, GP], bf16, kind="ExternalInput")
    bhhnD = nc.dram_tensor("bhhnD", [128, 96], bf16, kind="ExternalInput")
    wlrD = nc.dram_tensor("wlrD", [128, 6], bf16, kind="ExternalInput")
    wpkT = nc.dram_tensor("wpkT", [DP, DP], bf16, kind="ExternalInput")
    wmT = nc.dram_tensor("wmT", [DP, DP], bf16, kind="ExternalInput")
    wdT = nc.dram_tensor("wdT", [DP, 4], bf16, kind="ExternalInput")
    hv2 = nc.dram_tensor("hv2", [DP, 1], bf16, kind="ExternalInput")
    hv = nc.dram_tensor("hv", [DP, 1], bf16, kind="ExternalInput")
    identD = nc.dram_tensor("identD", [128, 128], bf16, kind="ExternalInput")
    bvecsD = nc.dram_tensor("bvecsD", [DP, 3], f32, kind="ExternalInput")
    bdD = nc.dram_tensor("bdD", [4, 1], f32, kind="ExternalInput")
    sigbD = nc.dram_tensor("sigbD", [SPB, 2], f32, kind="ExternalInput")
    rmlD = nc.dram_tensor("rmlD", [SPB, 1], f32, kind="ExternalInput")
    ralD = nc.dram_tensor("ralD", [1, SPB], f32, kind="ExternalInput")
    qacD = nc.dram_tensor("qacD", [SPB, 1], f32, kind="ExternalInput")
    masksD = nc.dram_tensor("masksD", [SPB, 3, L], bf16, kind="ExternalInput")
    gidxD = nc.dram_tensor("gidxD", [128, NG4], i32, kind="ExternalInput")
    midxD = nc.dram_tensor("midxD", [128, 64], i32, kind="ExternalInput")
    aidxD = nc.dram_tensor("aidxD", [128, 1], i32, kind="ExternalInput")
    smatD = nc.dram_tensor("smatD", [SPB, 128, 4, L], bf16, kind="ExternalInput")

    out_probs = nc.dram_tensor("out_probs", [SPB, 3], f32, kind="ExternalOutput")
    if debug:
        dbg_h = nc.dram_tensor("dbg_h", [128, 96], f32, kind="ExternalOutput")
        dbg_attn = nc.dram_tensor("dbg_attn", [SPB, 2, T], f32, kind="ExternalOutput")
        dbg_w = nc.dram_tensor("dbg_w", [SPB, L], f32, kind="ExternalOutput")

    with tile.TileContext(nc) as tc:
        outer = contextlib.ExitStack()
        with outer:
            con = outer.enter_context(tc.tile_pool(name="con", bufs=1))
            sb = outer.enter_context(tc.tile_pool(name="sb", bufs=1))

            # ---------------- constants ----------------
            ident = con.tile([128, 128], bf16)
            nc.sync.dma_start(ident[:], identD[:])
            wih_t = [[con.tile([128, GP], bf16, name=f"wih{s}{k}") for k in range(3)]
                     for s in range(2)]
            whh_t = [[con.tile([128, GP], bf16, name=f"whh{s}{k}") for k in range(3)]
                     for s in range(2)]
            for s, src_ih, src_hh in ((0, wih_l, whh_l), (1, wih_r, whh_r)):
                for k in range(3):
                    nc.sync.dma_start(wih_t[s][k][:], src_ih[128 * k:128 * (k + 1), :])
                    nc.sync.dma_start(whh_t[s][k][:], src_hh[128 * k:128 * (k + 1), :])
            bhhn_t = con.tile([128, 96], bf16)
            nc.sync.dma_start(bhhn_t[:], bhhnD[:])
            wlr_t = con.tile([128, 6], bf16)
            nc.sync.dma_start(wlr_t[:], wlrD[:])
            sigb_t = con.tile([SPB, 2], f32)
            nc.sync.dma_start(sigb_t[:], sigbD[:])
            gidx_t = con.tile([128, NG4], i32)
            nc.sync.dma_start(gidx_t[:], gidxD[:])
            midx_t = con.tile([128, 64], i32)
            nc.sync.dma_start(midx_t[:], midxD[:])
            aidx_t = con.tile([128, 1], i32)
            nc.sync.dma_start(aidx_t[:], aidxD[:])
            masks_t = con.tile([SPB, 3, L], bf16)
            nc.sync.dma_start(masks_t[:], masksD[:])
            wpkT_t = [con.tile([128, DP], bf16, name=f"wpkT{k}") for k in range(3)]
            wmT_t = [con.tile([128, DP], bf16, name=f"wmT{k}") for k in range(3)]
            wdT_t = [con.tile([128, 4], bf16, name=f"wdT{k}") for k in range(3)]
            hv2_t = [con.tile([128, 1], bf16, name=f"hv2{k}") for k in range(3)]
            hv_t = [con.tile([128, 1], bf16, name=f"hv{k}") for k in range(3)]
            bvec_t = [con.tile([128, 3], f32, name=f"bvec{k}") for k in range(3)]
            for k in range(3):
                o = 128 * k
                nc.sync.dma_start(wpkT_t[k][:], wpkT[o:o + 128, :])
                nc.sync.dma_start(wmT_t[k][:], wmT[o:o + 128, :])
                nc.sync.dma_start(wdT_t[k][:], wdT[o:o + 128, :])
                nc.sync.dma_start(hv2_t[k][:], hv2[o:o + 128, :])
                nc.sync.dma_start(hv_t[k][:], hv[o:o + 128, :])
                nc.sync.dma_start(bvec_t[k][:], bvecsD[o:o + 128, :])
            bd_t = con.tile([4, 1], f32)
            nc.sync.dma_start(bd_t[:], bdD[:])
            rml_t = con.tile([SPB, 1], f32)
            nc.sync.dma_start(rml_t[:], rmlD[:])
            ral_t = con.tile([SPB, 1], f32)
            nc.sync.dma_start(ral_t[:], ralD[:])
            qac_t = con.tile([SPB, 1], f32)
            nc.sync.dma_start(qac_t[:], qacD[:])


            # ---------------- persistent state ----------------
            # h_fm[p, 32c+16s+b] = h_{side s, seq b}[128c+p]  (c2 rows 44.. = 0)
            h_fm = sb.tile([128, 96], bf16)
            nc.vector.memset(h_fm[:], 0.0)
            attn_sb = sb.tile([SPB, 2, T], f32)      # h.wl / h.wr dots per step
            ex_sb = [sb.tile([128, 3, 128], bf16, name=f"ex{i}") for i in range(2)]
            stg_n = [sb.tile([128, 4, 96], bf16, name=f"sgn{i}") for i in range(2)]
            # post-phase memory embeddings, prepped during the GRU loop
            mrow_all = sb.tile([128, 64, DP], bf16)      # [p, 4b+q, dim]
            emT_all = sb.tile([128, SPB, 3, L], bf16)    # [dim_p, b, c, pos]

            # =============== GRU phase ===============
            gru = contextlib.ExitStack()
            with gru:
                work = gru.enter_context(tc.tile_pool(name="work", bufs=2))
                ps_g = gru.enter_context(tc.tile_pool(name="ps_g", bufs=2, space="PSUM"))
                ps_n = gru.enter_context(tc.tile_pool(name="ps_n", bufs=1, space="PSUM"))
                ps_e = gru.enter_context(tc.tile_pool(name="ps_e", bufs=1, space="PSUM"))
                ps_a = gru.enter_context(tc.tile_pool(name="ps_a", bufs=1, space="PSUM"))
                ps_gn = gru.enter_context(tc.tile_pool(name="ps_gn", bufs=1, space="PSUM"))

                # pg_rz[p, s4, 96g+32mc+16side+b]: r/z gate preacts, 4 steps
                pg_rz = [ps_g.tile([128, 4, 192], f32, name=f"pgrz{i}", tag="pgrz")
                         for i in range(2)]
                pg_n = ps_n.tile([128, 4, 96], f32, name="pgn", tag="pgn")
                ps_ex = ps_e.tile([128, 3, 128], bf16, name="psex", tag="psex")
                attn_ps = ps_a.tile([SPB, 2, 64], f32, name="attnps", tag="attnps")
                gates_n = ps_gn.tile([128, 96], f32, name="gatesn", tag="gatesn")

                def prod_gather(g):
                    rows = work.tile([128, DP], bf16, name="gthr", tag="gthr")
                    nc.gpsimd.indirect_dma_start(
                        out=rows[:], out_offset=None, in_=embp[:],
                        in_offset=bass.IndirectOffsetOnAxis(ap=gidx_t[:, g:g + 1], axis=0))
                    return rows

                def prod_transpose(g, rows):
                    for c in range(3):
                        nc.tensor.transpose(ps_ex[:, c, :], rows[:, 128 * c:128 * (c + 1)],
                                            ident[:])
                    nc.scalar.activation(ex_sb[g % 2][:], ps_ex[:], AF.Copy)

                def prod_mm(g, part):
                    # gi matmuls: out[128, (4 steps, 16 b)] over 64 tokens of one side
                    ex = ex_sb[g % 2]
                    prz = pg_rz[g % 2]
                    tasks = [(gg, mc, s) for gg in range(3) for mc in range(3)
                             for s in range(2)]
                    lo, hi = (0, 9) if part == 0 else (9, 18)
                    for (gg, mc, s) in tasks[lo:hi]:
                        for kc in range(3):
                            if gg < 2:
                                out = prz[:, :, 96 * gg + 32 * mc + 16 * s:
                                          96 * gg + 32 * mc + 16 * s + 16]
                            else:
                                out = pg_n[:, :, 32 * mc + 16 * s:32 * mc + 16 * s + 16]
                            nc.tensor.matmul(
                                out,
                                lhsT=wih_t[s][kc][:, 384 * gg + 128 * mc:
                                                  384 * gg + 128 * (mc + 1)],
                                rhs=ex[:, kc, 64 * s:64 * s + 64],
                                start=(kc == 0), stop=False,
                                skip_group_check=True)

                def prod_ncopy(g):
                    nc.scalar.activation(stg_n[g % 2][:], pg_n[:], AF.Copy)

                def prep_gather(j):
                    nc.gpsimd.indirect_dma_start(
                        out=mrow_all[:, j, :], out_offset=None, in_=embp[:],
                        in_offset=bass.IndirectOffsetOnAxis(
                            ap=midx_t[:, j:j + 1], axis=0))

                def prep_tr(j):
                    b, q = j // 4, j % 4
                    for c in range(3):
                        nc.tensor.transpose(ps_ex[:, c, :],
                                            mrow_all[:, j, 128 * c:128 * (c + 1)],
                                            ident[:])
                    nc.scalar.activation(emT_all[:, b, :, 128 * q:128 * (q + 1)],
                                         ps_ex[:], AF.Copy)

                # preamble: produce group 0 fully; prefetch first mem gathers
                rows0 = prod_gather(0)
                prod_transpose(0, rows0)
                prod_mm(0, 0)
                prod_mm(0, 1)
                prod_ncopy(0)
                prep_gather(0)
                prep_gather(1)
                prep_slot = 0

                for t in range(T):
                    s4 = t % 4
                    gg = t // 4
                    gp = gg + 1     # group being produced during this window
                    prz = pg_rz[gg % 2]

                    # ---- recurrent matmuls (continue the gi groups) ----
                    for gate in range(2):           # r then z
                        for mc in range(3):
                            for side in range(2):
                                col = 96 * gate + 32 * mc + 16 * side
                                for kc in range(3):
                                    nc.tensor.matmul(
                                        prz[:, s4, col:col + 16],
                                        lhsT=whh_t[side][kc][:, 384 * gate + 128 * mc:
                                                             384 * gate + 128 * (mc + 1)],
                                        rhs=h_fm[:, 32 * kc + 16 * side:
                                                 32 * kc + 16 * side + 16],
                                        start=False, stop=(kc == 2),
                                        skip_group_check=True)
                    # n-gate: bhh_n init + Whh_n accumulation (no gi here)
                    for mc in range(3):
                        for side in range(2):
                            col = 32 * mc + 16 * side
                            nc.tensor.matmul(
                                gates_n[0:128, col:col + 16],
                                lhsT=ident[:, 0:128],
                                rhs=bhhn_t[:, col:col + 16],
                                start=True, stop=False, skip_group_check=True)
                            for kc in range(3):
                                nc.tensor.matmul(
                                    gates_n[0:128, col:col + 16],
                                    lhsT=whh_t[side][kc][:, 768 + 128 * mc:
                                                         768 + 128 * (mc + 1)],
                                    rhs=h_fm[:, 32 * kc + 16 * side:
                                             32 * kc + 16 * side + 16],
                                    start=False, stop=(kc == 2),
                                    skip_group_check=True)

                    # ---- gate nonlinearities / state update ----
                    r_sb = work.tile([128, 96], bf16, name="r_sb", tag="r_sb")
                    nc.scalar.activation(r_sb[:], prz[:, s4, 0:96], AF.Sigmoid)
                    zc_sb = work.tile([128, 96], bf16, name="zc_sb", tag="zc_sb")
                    nc.scalar.activation(zc_sb[:], prz[:, s4, 96:192], AF.Sigmoid,
                                         scale=-1.0)
                    t1 = work.tile([128, 96], bf16, name="t1", tag="t1")
                    nc.vector.tensor_tensor(out=t1[:], in0=r_sb[:], in1=gates_n[:],
                                            op=OP.mult)
                    t2 = work.tile([128, 96], bf16, name="t2", tag="t2")
                    nc.vector.tensor_tensor(out=t2[:], in0=t1[:],
                                            in1=stg_n[gg % 2][:, s4, :], op=OP.add)
                    n_sb = work.tile([128, 96], bf16, name="n_sb", tag="n_sb")
                    nc.scalar.activation(n_sb[:], t2[:], AF.Tanh)
                    ta = work.tile([128, 96], bf16, name="ta", tag="ta")
                    nc.vector.tensor_tensor(out=ta[:], in0=zc_sb[:], in1=h_fm[:],
                                            op=OP.mult)
                    tb = work.tile([128, 96], bf16, name="tb", tag="tb")
                    nc.vector.tensor_tensor(out=tb[:], in0=h_fm[:], in1=ta[:],
                                            op=OP.subtract)
                    tcm = work.tile([128, 96], bf16, name="tcm", tag="tcm")
                    nc.vector.tensor_tensor(out=tcm[:], in0=zc_sb[:], in1=n_sb[:],
                                            op=OP.mult)
                    nc.vector.tensor_tensor(out=h_fm[:], in0=tb[:], in1=tcm[:],
                                            op=OP.add)

                    # ---- attention dot: attn[b, side, t] = h_side_b . w_side ----
                    w64 = t % 64
                    for side in range(2):
                        for c in range(3):
                            nc.tensor.matmul(
                                attn_ps[:, side, w64:w64 + 1],
                                lhsT=h_fm[:, 32 * c + 16 * side:32 * c + 16 * side + 16],
                                rhs=wlr_t[:, 2 * c + side:2 * c + side + 1],
                                start=(c == 0), stop=(c == 2),
                                skip_group_check=True)
                    if w64 == 63 or t == T - 1:
                        t0c = t - w64
                        nc.scalar.activation(attn_sb[:, :, t0c:t + 1],
                                             attn_ps[:, :, 0:w64 + 1], AF.Copy)

                    # ---- spread production of group gp ----
                    if gp < NG4:
                        if s4 == 0:
                            work._prod_rows = prod_gather(gp)
                        elif s4 == 1:
                            prod_transpose(gp, work._prod_rows)
                        elif s4 == 2:
                            prod_mm(gp, 0)
                        else:
                            prod_mm(gp, 1)
                            prod_ncopy(gp)
                    # ---- post-phase memory prep (2 chunks / 4 steps) ----
                    if s4 in (0, 2) and prep_slot < 64:
                        prep_tr(prep_slot)
                        if prep_slot + 2 < 64:
                            prep_gather(prep_slot + 2)
                        prep_slot += 1

            if debug:
                dbgh = sb.tile([128, 96], f32)
                nc.vector.tensor_copy(dbgh[:], h_fm[:])
                nc.sync.dma_start(dbg_h[:], dbgh[:])
                nc.sync.dma_start(dbg_attn[:], attn_sb[:])

            # =============== post phase ===============
            post = contextlib.ExitStack()
            with post:
                pp = post.enter_context(tc.tile_pool(name="pp", bufs=1))
                pw = post.enter_context(tc.tile_pool(name="pw", bufs=2))
                ps_pa = post.enter_context(tc.tile_pool(name="ps_pa", bufs=2, space="PSUM"))
                ps_pb = post.enter_context(tc.tile_pool(name="ps_pb", bufs=1, space="PSUM"))
                ps_pc = post.enter_context(tc.tile_pool(name="ps_pc", bufs=2, space="PSUM"))

                # ---- attn sigmoid + 0.5 ----
                af = pp.tile([SPB, 2, L], bf16)
                nc.vector.memset(af[:], 0.0)
                af_f = pp.tile([SPB, 2, T], f32)
                for side in range(2):
                    nc.scalar.activation(af_f[:, side, :], attn_sb[:, side, :],
                                         AF.Sigmoid, bias=sigb_t[:, side:side + 1])
                nc.vector.tensor_scalar_add(af[:, :, 0:T], af_f[:], 0.5)

                # ---- arT[j, q, b] = af[b, 1, 128q+j] (for shifted attn_r) ----
                arT = pp.tile([128, 4, SPB], bf16)
                for q in range(4):
                    pta = ps_pa.tile([128, 128], bf16, name="pta", tag="pta")
                    nc.tensor.transpose(pta[0:128, 0:16], af[:, 1, 128 * q:128 * (q + 1)],
                                        ident[0:16, 0:16])
                    nc.scalar.activation(arT[:, q, :], pta[0:128, 0:16], AF.Copy)

                # ---- aspect -> score bias (qa + bk.wa1) ----
                arows = pp.tile([128, DP], bf16)
                nc.gpsimd.indirect_dma_start(
                    out=arows[:], out_offset=None, in_=embp[:],
                    in_offset=bass.IndirectOffsetOnAxis(ap=aidx_t[:, 0:1], axis=0))
                aspsum = pp.tile([128, 3 * SPB], f32)
                for c in range(3):
                    pta = ps_pa.tile([128, 128], bf16, name="pta2", tag="pta")
                    nc.tensor.transpose(pta[:], arows[:, 128 * c:128 * (c + 1)], ident[:])
                    aT = pw.tile([128, 128], f32, name="aT", tag="aT")
                    nc.vector.tensor_copy(aT[:], pta[:])
                    for b in range(SPB):
                        nc.vector.tensor_reduce(
                            aspsum[:, c * SPB + b:c * SPB + b + 1],
                            aT[:, 8 * b:8 * b + 5], axis=AX.X, op=OP.add)
                aspb = pp.tile([128, 3 * SPB], bf16)
                nc.vector.tensor_copy(aspb[:], aspsum[:])
                pqa = ps_pc.tile([SPB, SPB], f32, name="pqa", tag="small")
                for c in range(3):
                    nc.tensor.matmul(pqa[:, 0:1], lhsT=aspb[:, c * SPB:(c + 1) * SPB],
                                     rhs=hv_t[c][:, 0:1],
                                     start=(c == 0), stop=(c == 2))
                sc_bias = pp.tile([SPB, 1], f32)
                nc.vector.tensor_tensor(out=sc_bias[:], in0=pqa[:, 0:1], in1=ral_t[:],
                                        op=OP.mult)
                nc.vector.tensor_scalar_add(sc_bias[:], sc_bias[:], qac_t[:, 0:1])

                # ---- fused per-seq loop: s0 (from emT) + shifted attn_r (smat) ----
                s0_all = pp.tile([SPB, L], f32)
                wsh = pp.tile([SPB, L], bf16)
                for b in range(SPB):
                    ps0 = ps_pc.tile([1, L], f32, name="ps0", tag="ps0")
                    for c in range(3):
                        nc.tensor.matmul(ps0[:], lhsT=hv2_t[c][:, 0:1],
                                         rhs=emT_all[:, b, c, :],
                                         start=(c == 0), stop=(c == 2))
                    s0row = pw.tile([1, L], f32, name="s0row", tag="s0row")
                    nc.scalar.activation(s0row[:], ps0[:], AF.Copy)
                    nc.sync.dma_start(s0_all[b:b + 1, :], s0row[:])
                    smat_t = pw.tile([128, 4, L], bf16, name="smat_t", tag="smat")
                    nc.sync.dma_start(smat_t[:], smatD[b, :, :, :])
                    psh = ps_pc.tile([1, L], f32, name="psh", tag="ps0")
                    for q in range(4):
                        nc.tensor.matmul(psh[:], lhsT=arT[:, q:q + 1, b:b + 1],
                                         rhs=smat_t[:, q, :],
                                         start=(q == 0), stop=(q == 3))
                    shrow = pw.tile([1, L], bf16, name="shrow", tag="shrow")
                    nc.scalar.activation(shrow[:], psh[:], AF.Copy)
                    nc.sync.dma_start(wsh[b:b + 1, :], shrow[:])

                # ---- w combine ----
                w1 = pp.tile([SPB, L], bf16)
                nc.vector.tensor_tensor(out=w1[:], in0=af[:, 0, :], in1=masks_t[:, 0, :],
                                        op=OP.mult)
                w2 = pp.tile([SPB, L], bf16)
                nc.vector.tensor_tensor(out=w2[:], in0=wsh[:], in1=masks_t[:, 1, :],
                                        op=OP.mult)
                wt_ = pp.tile([SPB, L], bf16)
                nc.vector.tensor_tensor(out=wt_[:], in0=w1[:], in1=w2[:], op=OP.add)
                nc.vector.tensor_tensor(out=wt_[:], in0=wt_[:], in1=masks_t[:, 2, :],
                                        op=OP.add)
                w_n = pp.tile([SPB, L], bf16)
                nc.vector.tensor_scalar_mul(w_n[:], wt_[:], rml_t[:, 0:1])
                if debug:
                    dbgw = pp.tile([SPB, L], f32)
                    nc.vector.tensor_copy(dbgw[:], wt_[:])
                    nc.sync.dma_start(dbg_w[:], dbgw[:])

                # ---- wn column transposes (wnT[p, q, b] = w_n[b, 128q+p]) ----
                wnT = pp.tile([128, 4, SPB], bf16)
                for q in range(4):
                    ptw = ps_pa.tile([128, 128], bf16, name="ptw", tag="pta")
                    nc.tensor.transpose(ptw[0:128, 0:16], w_n[:, 128 * q:128 * (q + 1)],
                                        ident[0:16, 0:16])
                    nc.scalar.activation(wnT[:, q, :], ptw[0:128, 0:16], AF.Copy)

                # ---- batched score/softmax over [SPB, L] ----
                spre = pp.tile([SPB, L], f32)
                nc.vector.tensor_tensor(out=spre[:], in0=wt_[:], in1=s0_all[:],
                                        op=OP.mult)
                score = pp.tile([SPB, L], f32)
                nc.scalar.activation(score[:], spre[:], AF.Tanh, bias=sc_bias[:, 0:1])
                mneg = pp.tile([SPB, 1], f32)
                nc.vector.tensor_reduce(mneg[:], score[:], axis=AX.X, op=OP.max,
                                        negate=True)
                ex_t = pp.tile([SPB, L], f32)
                zsum = pp.tile([SPB, 1], f32)
                nc.scalar.activation(ex_t[:], score[:], AF.Exp, bias=mneg[:, 0:1],
                                     accum_out=zsum[:, 0:1])
                zrec = pp.tile([SPB, 1], f32)
                nc.vector.reciprocal(zrec[:], zsum[:])
                prob = pp.tile([SPB, L], f32)
                nc.vector.tensor_scalar_mul(prob[:], ex_t[:], zrec[:, 0:1])
                cv_all = pp.tile([SPB, L], bf16)
                nc.vector.tensor_tensor(out=cv_all[:], in0=prob[:], in1=wt_[:],
                                        op=OP.mult)
                # cv column form (cvT[p, q, b] = cv[b, 128q+p])
                cvT = pp.tile([128, 4, SPB], bf16)
                for q in range(4):
                    ptc = ps_pa.tile([128, 128], bf16, name="ptc", tag="pta")
                    nc.tensor.transpose(ptc[0:128, 0:16], cv_all[:, 128 * q:128 * (q + 1)],
                                        ident[0:16, 0:16])
                    nc.scalar.activation(cvT[:, q, :], ptc[0:128, 0:16], AF.Copy)

                # ---- mw / vs weighted sums via n=1 matmuls ----
                uts_ps = ps_pb.tile([128, 3 * SPB], f32, name="utsps", tag="utsps")
                vs_ps = ps_pb.tile([128, 3 * SPB], f32, name="vsps", tag="vsps")
                for b in range(SPB):
                    for c in range(3):
                        for q in range(4):
                            nc.tensor.matmul(
                                uts_ps[:, c * SPB + b:c * SPB + b + 1],
                                lhsT=mrow_all[:, 4 * b + q, 128 * c:128 * (c + 1)],
                                rhs=cvT[:, q, b:b + 1],
                                start=(q == 0), stop=(q == 3), skip_group_check=True)
                            nc.tensor.matmul(
                                vs_ps[:, c * SPB + b:c * SPB + b + 1],
                                lhsT=mrow_all[:, 4 * b + q, 128 * c:128 * (c + 1)],
                                rhs=wnT[:, q, b:b + 1],
                                start=(q == 0), stop=(q == 3), skip_group_check=True)

                # ---- Wpk.mw + bpk + v_s -> Wm/tanh -> Wd -> softmax ----
                utsb = pp.tile([128, 3 * SPB], bf16)
                nc.vector.tensor_copy(utsb[:], uts_ps[:])
                vsT = pp.tile([128, 3 * SPB], f32)
                nc.scalar.activation(vsT[:], vs_ps[:], AF.Copy)
                vns = pp.tile([128, 3 * SPB], bf16)
                for oc in range(3):
                    pv = ps_pc.tile([128, SPB], f32, name="pv", tag="small")
                    for ic in range(3):
                        nc.tensor.matmul(pv[:],
                                         lhsT=wpkT_t[ic][:, 128 * oc:128 * (oc + 1)],
                                         rhs=utsb[:, ic * SPB:(ic + 1) * SPB],
                                         start=(ic == 0), stop=(ic == 2))
                    nc.vector.scalar_tensor_tensor(
                        out=vns[:, oc * SPB:(oc + 1) * SPB], in0=pv[:],
                        scalar=bvec_t[oc][:, 1:2], in1=vsT[:, oc * SPB:(oc + 1) * SPB],
                        op0=OP.add, op1=OP.add)
                vms = pp.tile([128, 3 * SPB], bf16)
                for oc in range(3):
                    pv2 = ps_pc.tile([128, SPB], f32, name="pv2", tag="small")
                    for ic in range(3):
                        nc.tensor.matmul(pv2[:],
                                         lhsT=wmT_t[ic][:, 128 * oc:128 * (oc + 1)],
                                         rhs=vns[:, ic * SPB:(ic + 1) * SPB],
                                         start=(ic == 0), stop=(ic == 2))
                    nc.scalar.activation(vms[:, oc * SPB:(oc + 1) * SPB], pv2[:],
                                         AF.Tanh, bias=bvec_t[oc][:, 2:3])
                plg = ps_pc.tile([4, SPB], f32, name="plg", tag="small")
                for ic in range(3):
                    nc.tensor.matmul(plg[:], lhsT=wdT_t[ic][:, 0:4],
                                     rhs=vms[:, ic * SPB:(ic + 1) * SPB],
                                     start=(ic == 0), stop=(ic == 2))
                lgb = pp.tile([4, SPB], bf16)
                nc.vector.tensor_scalar_add(lgb[:], plg[:], bd_t[0:4, 0:1])
                plt = ps_pc.tile([SPB, 4], bf16, name="plt", tag="small")
                nc.tensor.matmul(plt[:], lhsT=lgb[:], rhs=ident[0:4, 0:4],
                                 start=True, stop=True, is_transpose=True)
                mneg2 = pp.tile([SPB, 1], f32)
                nc.vector.tensor_reduce(mneg2[:], plt[:, 0:3], axis=AX.X, op=OP.max,
                                        negate=True)
                ex2 = pp.tile([SPB, 3], f32)
                z2 = pp.tile([SPB, 1], f32)
                nc.scalar.activation(ex2[:], plt[:, 0:3], AF.Exp, bias=mneg2[:, 0:1],
                                     accum_out=z2[:, 0:1])
                z2r = pp.tile([SPB, 1], f32)
                nc.vector.reciprocal(z2r[:], z2[:])
                res = pp.tile([SPB, 3], f32)
                nc.vector.tensor_scalar_mul(res[:], ex2[:], z2r[:, 0:1])
                nc.sync.dma_start(out_probs[:], res[:])

    nc.compile()
    return nc


def _host_prep(inputs, T_override=None):
    bf = ml_dtypes.bfloat16
    emb = np.asarray(inputs['embedding'], np.float32)
    ti = np.asarray(inputs['text_raw_indices'])
    ai = np.asarray(inputs['aspect_indices'])
    xl = np.asarray(inputs['x_l'])
    xr = np.asarray(inputs['x_r'])
    mem_len = (ti != 0).sum(-1).astype(np.int64)
    asp_len = (ai != 0).sum(-1).astype(np.int64)
    left_len = (xl != 0).sum(-1).astype(np.int64)
    right_len = (xr != 0).sum(-1).astype(np.int64)
    T = int(max(left_len.max(), right_len.max()))
    if T_override is not None:
        T = T_override

    embp = np.zeros((V, DP), np.float32)
    embp[:, :D] = emb
    embp[:, D] = 1.0
    embp[0, D + 1] = 1.0          # pad-token flag -> z-gate kill
    embp = embp.astype(bf)

    LARGE = 64.0

    def aug_ih(Wih, bih, bhh):
        # [DP, GP]: row i<300 = Wih^T; row 300 = bih (+bhh for r/z);
        # row 301 = +LARGE on z block (pad-token carry)
        Wih = np.asarray(Wih, np.float32)
        bih = np.asarray(bih, np.float32)
        bhh = np.asarray(bhh, np.float32)
        a = np.zeros((DP, GP), np.float32)
        for g in range(3):
            a[:D, 384 * g:384 * g + D] = Wih[D * g:D * (g + 1), :].T
            bb = bih[D * g:D * (g + 1)].copy()
            if g < 2:
                bb += bhh[D * g:D * (g + 1)]
            a[D, 384 * g:384 * g + D] = bb
        a[D + 1, 384:384 + D] = LARGE
        return a.astype(bf)

    def aug_hh(Whh):
        Whh = np.asarray(Whh, np.float32)
        a = np.zeros((DP, GP), np.float32)
        for g in range(3):
            a[:D, 384 * g:384 * g + D] = Whh[D * g:D * (g + 1), :].T
        return a.astype(bf)

    def padT(Wsq):
        a = np.zeros((DP, DP), np.float32)
        a[:D, :D] = np.asarray(Wsq, np.float32).T
        return a.astype(bf)

    bhhn = np.zeros((128, 96), np.float32)
    for side, key in ((0, 'bhh_l'), (1, 'bhh_r')):
        bh = np.asarray(inputs[key], np.float32)[2 * D:3 * D]
        for c in range(3):
            nrows = min(128, D - 128 * c)
            for bcol in range(SPB):
                bhhn[:nrows, 32 * c + 16 * side + bcol] = bh[128 * c:128 * c + nrows]
    wlr = np.zeros((128, 6), np.float32)
    for side, key in ((0, 'wl'), (1, 'wr')):
        wv = np.asarray(inputs[key], np.float32)[0]
        for c in range(3):
            nrows = min(128, D - 128 * c)
            wlr[:nrows, 2 * c + side] = wv[128 * c:128 * c + nrows]

    wa = np.asarray(inputs['w_att'], np.float32)
    wdT = np.zeros((DP, 4), np.float32)
    wdT[:D, :3] = np.asarray(inputs['Wd'], np.float32).T
    Wk_f = np.asarray(inputs['Wk'], np.float32)
    Wproj_f = np.asarray(inputs['Wproj'], np.float32)
    bk_f = np.asarray(inputs['bk'], np.float32)
    bvecs = np.zeros((DP, 3), np.float32)
    bvecs[:D, 1] = Wproj_f @ bk_f + np.asarray(inputs['bproj'], np.float32)
    bvecs[:D, 2] = np.asarray(inputs['bm'], np.float32)
    shared = {
        'embp': embp,
        'wih_l': aug_ih(inputs['Wih_l'], inputs['bih_l'], inputs['bhh_l']),
        'wih_r': aug_ih(inputs['Wih_r'], inputs['bih_r'], inputs['bhh_r']),
        'whh_l': aug_hh(inputs['Whh_l']),
        'whh_r': aug_hh(inputs['Whh_r']),
        'bhhnD': bhhn.astype(bf),
        'wlrD': wlr.astype(bf),
        'wpkT': padT(Wproj_f @ Wk_f),
        'wmT': padT(inputs['Wm']),
        'wdT': wdT.astype(bf),
        'hv2': np.concatenate([Wk_f.T @ wa[:D],
                               np.zeros(DP - D, np.float32)])[:, None].astype(bf),
        'hv': np.concatenate([np.asarray(inputs['Wq'], np.float32).T @ wa[D:],
                              np.zeros(DP - D, np.float32)])[:, None].astype(bf),
        'identD': np.eye(128, dtype=np.float32).astype(bf),
        'bvecsD': bvecs,
        'bdD': np.concatenate([np.asarray(inputs['bd'], np.float32),
                               [0.0]])[:, None].astype(np.float32),
    }
    qa_c = float(np.asarray(inputs['bq'], np.float32) @ wa[D:]) + \
        float(np.asarray(inputs['bk'], np.float32) @ wa[:D])

    NG4 = (T + 3) // 4
    per_core = []
    for c in range(NCORES):
        sl = slice(c * SPB, (c + 1) * SPB)
        xlc, xrc = xl[sl], xr[sl]
        mlc, alc = mem_len[sl], asp_len[sl]
        llc = left_len[sl]
        a_start = (llc - alc).astype(np.int64)

        gidx = np.zeros((128, NG4), np.int32)
        for g in range(NG4):
            for s in range(4):
                t = 4 * g + s
                if t >= T:
                    continue
                gidx[16 * s:16 * s + 16, g] = xlc[:, t]
                gidx[64 + 16 * s:64 + 16 * s + 16, g] = xrc[:, t]
        midx = np.zeros((128, 64), np.int32)
        for b in range(SPB):
            for q in range(4):
                midx[:, 4 * b + q] = ti[sl][b, 128 * q:128 * (q + 1)]
        aidx = np.zeros((128, 1), np.int32)
        for b in range(SPB):
            aidx[8 * b:8 * b + 5, 0] = ai[sl][b, :]

        idxL = np.arange(L)[None, :]
        mL = (idxL < llc[:, None]).astype(np.float32)
        mR = ((idxL >= a_start[:, None]) & (idxL < mlc[:, None])).astype(np.float32)
        mP = (idxL >= mlc[:, None]).astype(np.float32)
        masks = np.stack([mL, mR, mP], axis=1).astype(bf)

        smat = np.zeros((SPB, 128, 4, L), np.float32)
        for b in range(SPB):
            s = int(a_start[b])
            jj = np.arange(L - s)
            smat[b, jj % 128, jj // 128, jj + s] = 1.0
        sig_b = np.zeros((SPB, 2), np.float32)
        sig_b[:, 0] = float(np.asarray(inputs['bl'])[0])
        sig_b[:, 1] = float(np.asarray(inputs['br'])[0])

        pc = dict(shared)
        pc.update({
            'gidxD': gidx, 'midxD': midx, 'aidxD': aidx,
            'masksD': masks,
            'sigbD': sig_b,
            'rmlD': (1.0 / mlc.astype(np.float32))[:, None],
            'ralD': (1.0 / alc.astype(np.float32))[:, None],
            'qacD': np.full((SPB, 1), qa_c, np.float32),
            'smatD': smat.astype(bf),
        })
        per_core.append(pc)
    return T, per_core


def kernel(**inputs):
    from concourse.bass_utils import run_bass_kernel_spmd
    T, per_core = _host_prep(inputs)
    key = ("v2", T)
    if key not in _CACHE:
        _CACHE[key] = _build(T, debug=False)
    nc = _CACHE[key]
    res = run_bass_kernel_spmd(nc, per_core, list(range(NCORES)))
    out = np.zeros((B, NP), np.float32)
    for c in range(NCORES):
        out[c * SPB:(c + 1) * SPB, :] = res.results[c]["out_probs"]
    return out
